# revision 6
# baseline (speedup 1.0000x reference)
"""MemNN (end-to-end memory network) Trainium2 kernel.

All the heavy FLOPs are six (B*L, V) @ (V, D) embedding matmuls sharing
`facts` as LHS (A_h = facts @ Wa[h], C_h = facts @ Wc[h]), fused into one
(3200, 10000) @ (10000, 1536) matmul independent of the hop recurrence.

Sharding: vocab (contraction) split 8 ways; each core computes a partial
product, host sums the 8 partials and runs the tiny hop recurrence.

Precision: the fused matmul runs in bf16.  Measured end-to-end error is
~2e-3 against the 2e-2 tolerance (fp32r is 1.9e-4 but runs the PE at only
1 moving row/cycle; bf16 runs 2 rows/cycle on TRN2 -- measured 200 cycles
per 400-row matmul instruction -- which fp8+DoubleRow merely matches at
twice the instruction count per FLOP).  The tiny question embedding
(0.3% of FLOPs) stays exact fp32r in the kernel tail.
"""

import os

os.environ.setdefault("MYCRO_LOCAL_CACHE", "1")

import ml_dtypes
import numpy as np

import concourse.bass as bass
import concourse.mybir as mybir
import concourse.tile as tile
from concourse.bass_utils import run_bass_kernel_spmd

HOPS, B, L, V, D = 3, 64, 50, 10000, 256
NCORES = 8
BL = B * L                # 3200 moving rows
NF = 2 * HOPS * D         # 1536 fused output cols: [Wa0|Wa1|Wa2|Wc0|Wc1|Wc2]
VSH = V // NCORES         # 1250 vocab rows per core
KT = 10                   # contraction tiles of 128 per core
VPAD = KT * 128           # 1280 (zero-padded)
MCH = 400                 # moving-col chunk
NN = NF // 128            # 12 stationary W tiles
F32R = mybir.dt.float32r
F32 = mybir.dt.float32
BF16 = mybir.dt.bfloat16
NP_BF16 = ml_dtypes.bfloat16

_nc_cache = None
_last_result = None       # BassKernelResults of the most recent run (for profiling)


def _legalize_sync(nc):
    """Split multi-wait sync_info into standalone single-wait EventSemaphores.

    The walrus build in this environment enforces the raw-bass contract of at
    most ONE SyncWait per instruction ("Too many sync wait commands" in
    setupSyncWait otherwise), while Tile attaches every needed wait to the
    consuming instruction.  Hoisting all-but-one wait onto preceding
    InstEventSemaphore instructions on the same engine queue is semantically
    identical: engine queues are in-order, so a preceding wait blocks the
    queue exactly like an attached wait.  Updates are left untouched (they
    fire at completion and cannot be hoisted).
    """
    for func in nc.m.functions:
        for block in func.blocks:
            insts = list(block.instructions)
            out = []
            n = 0
            for inst in insts:
                si = inst.sync_info
                if si is not None and len(si.on_wait) > 1:
                    waits = list(si.on_wait)
                    for w in waits[:-1]:
                        ev = mybir.InstEventSemaphore(
                            name=f"{inst.name}-hoistw{n}", ins=[], outs=[]
                        )
                        n += 1
                        ev.engine = inst.engine
                        ev.sync_info = mybir.SyncInfo(on_wait=[w], on_update=[])
                        nc.register_instruction(ev)
                        out.append(ev)
                    inst.sync_info = mybir.SyncInfo(
                        on_wait=[waits[-1]], on_update=list(si.on_update)
                    )
                out.append(inst)
            if len(out) != len(insts):
                block.instructions = out
    return nc


_WIDTHS = [MCH] * (BL // MCH)
_STARTS = [sum(_WIDTHS[:i]) for i in range(len(_WIDTHS))]
assert sum(_WIDTHS) == BL


def _build(reps=1):
    """Build the SPMD device program.

    reps>1 repeats the main loop body (same data, same output addresses) --
    used only by the benchmark harness to measure device time differentially
    (per-call dispatch noise over the axon tunnel is ~ms, device time is
    ~100 us, so wall-clocking one launch cannot resolve it).
    """
    nc = bass.Bass(trn_type="TRN2")
    facts_b = nc.dram_tensor("facts_b", [VPAD, BL], BF16, kind="ExternalInput")
    wac_b = nc.dram_tensor("wac_b", [VPAD, NF], BF16, kind="ExternalInput")
    q_t = nc.dram_tensor("q_t", [VPAD, B], F32R, kind="ExternalInput")
    wq = nc.dram_tensor("wq", [VPAD, D], F32R, kind="ExternalInput")
    pac_t = nc.dram_tensor("pac_t", [NF, BL], F32, kind="ExternalOutput")
    pu = nc.dram_tensor("pu", [B, D], F32, kind="ExternalOutput")

    fr = facts_b.rearrange("(k p) n -> p k n", p=128)
    wr = wac_b.rearrange("(k p) n -> p k n", p=128)
    qr = q_t.rearrange("(k p) n -> p k n", p=128)
    wqr = wq.rearrange("(k p) n -> p k n", p=128)
    wmax = max(_WIDTHS)

    with (
        tile.TileContext(nc) as tc,
        tc.tile_pool(name="wpool", bufs=1) as wpool,
        tc.tile_pool(name="xpool", bufs=3) as xpool,
        tc.tile_pool(name="opool", bufs=6) as opool,
        tc.tile_pool(name="pspool", bufs=7, space="PSUM") as pspool,
    ):
        # Prologue DMA order: first wac n-slice 0 + first facts chunk (the
        # first matmul group's deps), then the rest of wac, then the small
        # question tensors.
        wt = wpool.tile([128, KT, NF], BF16)
        nc.sync.dma_start(wt[:, :, 0:128], wr[:, :, 0:128])
        xts = {}
        xts[0] = xpool.tile(
            [128, KT, _WIDTHS[0]], BF16, tag="xt", name="xt",
            padded_shape=[128, KT, wmax],
        )
        nc.sync.dma_start(xts[0][:], fr[:, :, 0 : _WIDTHS[0]])
        for off in range(128, NF, 512):
            end = min(off + 512, NF)
            nc.sync.dma_start(wt[:, :, off:end], wr[:, :, off:end])
        qtile = wpool.tile([128, KT, B], F32R)
        nc.sync.dma_start(qtile[:], qr)
        wqt = wpool.tile([128, KT, D], F32R)
        nc.sync.dma_start(wqt[:], wqr)

        def get_xt(mi):
            if mi not in xts:
                xts[mi] = xpool.tile(
                    [128, KT, _WIDTHS[mi]], BF16, tag="xt", name="xt",
                    padded_shape=[128, KT, wmax],
                )
                nc.sync.dma_start(
                    xts[mi][:], fr[:, :, _STARTS[mi] : _STARTS[mi] + _WIDTHS[mi]]
                )
            return xts[mi]

        # Main fused matmul: out(n, m) += sum_k wac[k, n].T @ facts_b[k, m]
        for _ in range(reps):
            for mi in range(len(_WIDTHS)):
                xt = get_xt(mi)
                for n in range(NN):
                    ps = pspool.tile(
                        [128, _WIDTHS[mi]], F32, tag="ps", name="ps",
                        padded_shape=[128, wmax],
                    )
                    for k in range(KT):
                        nc.tensor.matmul(
                            ps[:],
                            wt[:, k, n * 128 : (n + 1) * 128],
                            xt[:, k, :],
                            start=(k == 0),
                            stop=(k == KT - 1),
                        )
                    ot = opool.tile(
                        [128, _WIDTHS[mi]], F32, tag="ot", name="ot",
                        padded_shape=[128, wmax],
                    )
                    nc.vector.tensor_copy(ot[:], ps[:])
                    nc.sync.dma_start(
                        pac_t[
                            n * 128 : (n + 1) * 128,
                            _STARTS[mi] : _STARTS[mi] + _WIDTHS[mi],
                        ],
                        ot[:],
                    )
            xts.clear()

        # Question embedding at the tail: its PE work (10 small matmuls)
        # overlaps the main loop's epilogue.
        psq = pspool.tile([B, D], F32, tag="psq", bufs=1)
        for k in range(KT):
            nc.tensor.matmul(
                psq[:], qtile[:, k, :], wqt[:, k, :], start=(k == 0), stop=(k == KT - 1)
            )
        uo = opool.tile([B, D], F32, tag="uo")
        nc.any.tensor_copy(out=uo[:], in_=psq[:])
        nc.sync.dma_start(pu[:, :], uo[:])
    return _legalize_sync(nc)


def _shard_inputs(facts, question, Wq, Wa, Wc):
    fx = np.ascontiguousarray(facts, dtype=np.float32).reshape(BL, V)
    fxb = fx.astype(NP_BF16)
    qx = np.asarray(question, dtype=np.float32).sum(axis=1)  # (B, V) bag-of-words
    Wq = np.asarray(Wq, dtype=np.float32)
    Wa = np.asarray(Wa, dtype=np.float32)
    Wc = np.asarray(Wc, dtype=np.float32)
    wac_full = np.concatenate(
        [Wa[0], Wa[1], Wa[2], Wc[0], Wc[1], Wc[2]], axis=1
    ).astype(NP_BF16)

    in_maps = []
    for c in range(NCORES):
        sl = slice(c * VSH, (c + 1) * VSH)
        fb = np.zeros((VPAD, BL), NP_BF16)
        fb[:VSH] = fxb[:, sl].T
        qt = np.zeros((VPAD, B), np.float32)
        qt[:VSH] = qx[:, sl].T
        ws = np.zeros((VPAD, NF), NP_BF16)
        ws[:VSH] = wac_full[sl]
        wqs = np.zeros((VPAD, D), np.float32)
        wqs[:VSH] = Wq[sl]
        in_maps.append({"facts_b": fb, "q_t": qt, "wac_b": ws, "wq": wqs})
    return in_maps


def _wait_for_devices(min_wait_attempts=10):
    """The axon terminal occasionally reports a transient bad topology
    ("terminal has 1 core"); poll until all 8 NeuronCores are visible."""
    import time as _time

    import jax

    for attempt in range(min_wait_attempts):
        try:
            if len(jax.devices()) >= NCORES:
                return
        except Exception:  # noqa: BLE001 - backend init failure is retryable
            try:
                jax.clear_backends()
            except Exception:  # noqa: BLE001
                pass
        _time.sleep(15.0)
    # fall through: let the run itself raise a descriptive error


def _run_with_retries(nc, in_maps, attempts=4):
    """run_bass_kernel_spmd with retries: the axon terminal occasionally
    reports transient failures (device wedged / NRT_EXEC_UNIT_UNRECOVERABLE /
    temporary topology glitches) that succeed on re-dispatch."""
    import time as _time

    last_exc = None
    for attempt in range(attempts):
        try:
            return run_bass_kernel_spmd(nc, in_maps, list(range(NCORES)))
        except Exception as e:  # noqa: BLE001 - retry any runtime failure
            last_exc = e
            if attempt < attempts - 1:
                _time.sleep(10.0 * (attempt + 1))
                _wait_for_devices(min_wait_attempts=4)
    raise last_exc


def kernel(facts, question, Wq, Wa, Wc, Ww, bw):
    global _nc_cache, _last_result
    _wait_for_devices(min_wait_attempts=8)
    in_maps = _shard_inputs(facts, question, Wq, Wa, Wc)
    if _nc_cache is None:
        _nc_cache = _build()
    _last_result = _run_with_retries(_nc_cache, in_maps)
    res = _last_result.results

    # Unshard: sum the 8 partial products of the vocab-sharded matmul.
    ac_t = res[0]["pac_t"].copy()
    u = res[0]["pu"].copy()
    for r in res[1:]:
        ac_t += r["pac_t"]
        u += r["pu"]

    # Sequential hop recurrence (tiny: ~30 MFLOP vs 98.3 GFLOP on device).
    Ww = np.asarray(Ww, dtype=np.float32)
    bw = np.asarray(bw, dtype=np.float32)
    for h in range(HOPS):
        A = ac_t[h * D : (h + 1) * D].reshape(D, B, L)
        C = ac_t[(HOPS + h) * D : (HOPS + h + 1) * D].reshape(D, B, L)
        match = np.einsum("dbl,bd->bl", A, u)
        mm = match - match.max(axis=-1, keepdims=True)
        e = np.exp(mm)
        p = e / e.sum(axis=-1, keepdims=True)
        att = np.einsum("bl,dbl->bd", p, C)
        z = (u + att) @ Ww[h] + bw[h]
        if h == HOPS - 1:
            zz = z - z.max(axis=-1, keepdims=True)
            ez = np.exp(zz)
            u = ez / ez.sum(axis=-1, keepdims=True)
        else:
            u = np.maximum(z, 0.0)
    return np.ascontiguousarray(u, dtype=np.float32)


# revision 7
# speedup vs baseline: 1.2443x; 1.2443x over previous
"""MemNN (end-to-end memory network) Trainium2 kernel.

All the heavy FLOPs are six (B*L, V) @ (V, D) embedding matmuls sharing
`facts` as LHS (A_h = facts @ Wa[h], C_h = facts @ Wc[h]), fused into one
(3200, 10000) @ (10000, 1536) matmul independent of the hop recurrence.

Sharding: vocab (contraction) split 8 ways; each core computes a partial
product, host sums the 8 partials and runs the tiny hop recurrence.

Precision: the fused matmul runs in bf16 with bf16 partial outputs
(end-to-end error ~2.2e-3 vs the 2e-2 tolerance; fp32r is 1.9e-4 but
streams only 1 moving row/cycle where bf16 streams 2).

Schedule: moving chunks are 512 wide (the PSUM/moving-AP limit) and are
processed in blocks of 3 with the k/n loops outside the block, so each
128x128 stationary tile is reused by 3 consecutive matmuls: a bf16
weight load (~256 cycles) then hides under 3x256 cycles of streaming.
At 400-wide single-chunk order (one load per matmul) the measured cost
was 360 cycles/instruction; the pure-stream floor is 256 for 512 rows.
bf16 partial outputs keep total DMA (13.7 MB in + 9.8 MB out per core)
under the PE time so the pass stays PE-bound.

The tiny question embedding (0.3% of FLOPs) stays exact fp32r in the
kernel tail.
"""

import os

os.environ.setdefault("MYCRO_LOCAL_CACHE", "1")

import ml_dtypes
import numpy as np

import concourse.bass as bass
import concourse.mybir as mybir
import concourse.tile as tile
from concourse.bass_utils import run_bass_kernel_spmd

HOPS, B, L, V, D = 3, 64, 50, 10000, 256
NCORES = 8
BL = B * L                # 3200 moving rows
NF = 2 * HOPS * D         # 1536 fused output cols: [Wa0|Wa1|Wa2|Wc0|Wc1|Wc2]
VSH = V // NCORES         # 1250 vocab rows per core
KT = 10                   # contraction tiles of 128 per core
VPAD = KT * 128           # 1280 (zero-padded)
NN = NF // 128            # 12 stationary W tiles
F32R = mybir.dt.float32r
F32 = mybir.dt.float32
BF16 = mybir.dt.bfloat16
NP_BF16 = ml_dtypes.bfloat16

_nc_cache = None
_last_result = None       # BassKernelResults of the most recent run (for profiling)


def _legalize_sync(nc):
    """Split multi-wait sync_info into standalone single-wait EventSemaphores.

    The walrus build in this environment enforces the raw-bass contract of at
    most ONE SyncWait per instruction ("Too many sync wait commands" in
    setupSyncWait otherwise), while Tile attaches every needed wait to the
    consuming instruction.  Hoisting all-but-one wait onto preceding
    InstEventSemaphore instructions on the same engine queue is semantically
    identical: engine queues are in-order, so a preceding wait blocks the
    queue exactly like an attached wait.  Updates are left untouched (they
    fire at completion and cannot be hoisted).
    """
    for func in nc.m.functions:
        for block in func.blocks:
            insts = list(block.instructions)
            out = []
            n = 0
            for inst in insts:
                si = inst.sync_info
                if si is not None and len(si.on_wait) > 1:
                    waits = list(si.on_wait)
                    for w in waits[:-1]:
                        ev = mybir.InstEventSemaphore(
                            name=f"{inst.name}-hoistw{n}", ins=[], outs=[]
                        )
                        n += 1
                        ev.engine = inst.engine
                        ev.sync_info = mybir.SyncInfo(on_wait=[w], on_update=[])
                        nc.register_instruction(ev)
                        out.append(ev)
                    inst.sync_info = mybir.SyncInfo(
                        on_wait=[waits[-1]], on_update=list(si.on_update)
                    )
                out.append(inst)
            if len(out) != len(insts):
                block.instructions = out
    return nc


_WIDTHS = [512] * 6 + [128]
_STARTS = [sum(_WIDTHS[:i]) for i in range(len(_WIDTHS))]
_BLOCKS = [(0, 1, 2), (3, 4, 5), (6,)]
assert sum(_WIDTHS) == BL


def _build(reps=1):
    """Build the SPMD device program.

    reps>1 repeats the main loop body (same data, same output addresses) --
    used only by the benchmark harness to measure device time differentially
    (per-call dispatch noise over the axon tunnel is ~ms, device time is
    ~100 us, so wall-clocking one launch cannot resolve it).
    """
    nc = bass.Bass(trn_type="TRN2")
    facts_b = nc.dram_tensor("facts_b", [VPAD, BL], BF16, kind="ExternalInput")
    wac_b = nc.dram_tensor("wac_b", [VPAD, NF], BF16, kind="ExternalInput")
    q_t = nc.dram_tensor("q_t", [VPAD, B], F32R, kind="ExternalInput")
    wq = nc.dram_tensor("wq", [VPAD, D], F32R, kind="ExternalInput")
    pac_t = nc.dram_tensor("pac_t", [NF, BL], BF16, kind="ExternalOutput")
    pu = nc.dram_tensor("pu", [B, D], F32, kind="ExternalOutput")

    fr = facts_b.rearrange("(k p) n -> p k n", p=128)
    wr = wac_b.rearrange("(k p) n -> p k n", p=128)
    qr = q_t.rearrange("(k p) n -> p k n", p=128)
    wqr = wq.rearrange("(k p) n -> p k n", p=128)
    wmax = max(_WIDTHS)

    with (
        tile.TileContext(nc) as tc,
        tc.tile_pool(name="wpool", bufs=1) as wpool,
        tc.tile_pool(name="xpool", bufs=6) as xpool,
        tc.tile_pool(name="opool", bufs=6) as opool,
        tc.tile_pool(name="pspool", bufs=6, space="PSUM") as pspool,
    ):
        # Prologue DMA order: first wac n-slice 0 + block 0's facts chunks
        # (the first matmul group's deps), then the rest of wac, then the
        # small question tensors.
        wt = wpool.tile([128, KT, NF], BF16)
        nc.sync.dma_start(wt[:, :, 0:128], wr[:, :, 0:128])
        xts = {}

        def get_xt(mi):
            if mi not in xts:
                xts[mi] = xpool.tile(
                    [128, KT, _WIDTHS[mi]], BF16, tag="xt", name="xt",
                    padded_shape=[128, KT, wmax],
                )
                nc.sync.dma_start(
                    xts[mi][:], fr[:, :, _STARTS[mi] : _STARTS[mi] + _WIDTHS[mi]]
                )
            return xts[mi]

        get_xt(0)
        for off in range(128, NF, 512):
            end = min(off + 512, NF)
            nc.sync.dma_start(wt[:, :, off:end], wr[:, :, off:end])
        qtile = wpool.tile([128, KT, B], F32R)
        nc.sync.dma_start(qtile[:], qr)
        wqt = wpool.tile([128, KT, D], F32R)
        nc.sync.dma_start(wqt[:], wqr)

        # Main fused matmul: out(n, m) += sum_k wac[k, n].T @ facts_b[k, m].
        # Within each 3-chunk block the k/n loops are outside the chunk loop,
        # so each stationary tile serves 3 back-to-back matmuls and its load
        # hides under the previous instruction's 256-cycle stream.
        for _ in range(reps):
            for blk in _BLOCKS:
                xbs = [get_xt(mi) for mi in blk]
                for n in range(NN):
                    pss = [
                        pspool.tile(
                            [128, _WIDTHS[mi]], F32, tag="ps", name="ps",
                            padded_shape=[128, wmax],
                        )
                        for mi in blk
                    ]
                    for k in range(KT):
                        for j, mi in enumerate(blk):
                            nc.tensor.matmul(
                                pss[j][:],
                                wt[:, k, n * 128 : (n + 1) * 128],
                                xbs[j][:, k, :],
                                start=(k == 0),
                                stop=(k == KT - 1),
                            )
                    for j, mi in enumerate(blk):
                        ot = opool.tile(
                            [128, _WIDTHS[mi]], BF16, tag="ot", name="ot",
                            padded_shape=[128, wmax],
                        )
                        nc.vector.tensor_copy(ot[:], pss[j][:])
                        nc.sync.dma_start(
                            pac_t[
                                n * 128 : (n + 1) * 128,
                                _STARTS[mi] : _STARTS[mi] + _WIDTHS[mi],
                            ],
                            ot[:],
                        )
            xts.clear()

        # Question embedding at the tail: its PE work (10 small matmuls)
        # overlaps the main loop's epilogue.
        psq = pspool.tile([B, D], F32, tag="psq", bufs=1)
        for k in range(KT):
            nc.tensor.matmul(
                psq[:], qtile[:, k, :], wqt[:, k, :], start=(k == 0), stop=(k == KT - 1)
            )
        uo = opool.tile([B, D], F32, tag="uo")
        nc.any.tensor_copy(out=uo[:], in_=psq[:])
        nc.sync.dma_start(pu[:, :], uo[:])
    return _legalize_sync(nc)


def _shard_inputs(facts, question, Wq, Wa, Wc):
    fx = np.ascontiguousarray(facts, dtype=np.float32).reshape(BL, V)
    fxb = fx.astype(NP_BF16)
    qx = np.asarray(question, dtype=np.float32).sum(axis=1)  # (B, V) bag-of-words
    Wq = np.asarray(Wq, dtype=np.float32)
    Wa = np.asarray(Wa, dtype=np.float32)
    Wc = np.asarray(Wc, dtype=np.float32)
    wac_full = np.concatenate(
        [Wa[0], Wa[1], Wa[2], Wc[0], Wc[1], Wc[2]], axis=1
    ).astype(NP_BF16)

    in_maps = []
    for c in range(NCORES):
        sl = slice(c * VSH, (c + 1) * VSH)
        fb = np.zeros((VPAD, BL), NP_BF16)
        fb[:VSH] = fxb[:, sl].T
        qt = np.zeros((VPAD, B), np.float32)
        qt[:VSH] = qx[:, sl].T
        ws = np.zeros((VPAD, NF), NP_BF16)
        ws[:VSH] = wac_full[sl]
        wqs = np.zeros((VPAD, D), np.float32)
        wqs[:VSH] = Wq[sl]
        in_maps.append({"facts_b": fb, "q_t": qt, "wac_b": ws, "wq": wqs})
    return in_maps


def _wait_for_devices(min_wait_attempts=10):
    """The axon terminal occasionally reports a transient bad topology
    ("terminal has 1 core"); poll until all 8 NeuronCores are visible."""
    import time as _time

    import jax

    for attempt in range(min_wait_attempts):
        try:
            if len(jax.devices()) >= NCORES:
                return
        except Exception:  # noqa: BLE001 - backend init failure is retryable
            try:
                jax.clear_backends()
            except Exception:  # noqa: BLE001
                pass
        _time.sleep(15.0)
    # fall through: let the run itself raise a descriptive error


def _run_with_retries(nc, in_maps, attempts=4):
    """run_bass_kernel_spmd with retries: the axon terminal occasionally
    reports transient failures (device wedged / NRT_EXEC_UNIT_UNRECOVERABLE /
    temporary topology glitches) that succeed on re-dispatch."""
    import time as _time

    last_exc = None
    for attempt in range(attempts):
        try:
            return run_bass_kernel_spmd(nc, in_maps, list(range(NCORES)))
        except Exception as e:  # noqa: BLE001 - retry any runtime failure
            last_exc = e
            if attempt < attempts - 1:
                _time.sleep(10.0 * (attempt + 1))
                _wait_for_devices(min_wait_attempts=4)
    raise last_exc


def kernel(facts, question, Wq, Wa, Wc, Ww, bw):
    global _nc_cache, _last_result
    _wait_for_devices(min_wait_attempts=8)
    in_maps = _shard_inputs(facts, question, Wq, Wa, Wc)
    if _nc_cache is None:
        _nc_cache = _build()
    _last_result = _run_with_retries(_nc_cache, in_maps)
    res = _last_result.results

    # Unshard: sum the 8 bf16 partial products of the vocab-sharded matmul.
    ac_t = res[0]["pac_t"].astype(np.float32)
    u = res[0]["pu"].copy()
    for r in res[1:]:
        ac_t += r["pac_t"].astype(np.float32)
        u += r["pu"]

    # Sequential hop recurrence (tiny: ~30 MFLOP vs 98.3 GFLOP on device).
    Ww = np.asarray(Ww, dtype=np.float32)
    bw = np.asarray(bw, dtype=np.float32)
    for h in range(HOPS):
        A = ac_t[h * D : (h + 1) * D].reshape(D, B, L)
        C = ac_t[(HOPS + h) * D : (HOPS + h + 1) * D].reshape(D, B, L)
        match = np.einsum("dbl,bd->bl", A, u)
        mm = match - match.max(axis=-1, keepdims=True)
        e = np.exp(mm)
        p = e / e.sum(axis=-1, keepdims=True)
        att = np.einsum("bl,dbl->bd", p, C)
        z = (u + att) @ Ww[h] + bw[h]
        if h == HOPS - 1:
            zz = z - z.max(axis=-1, keepdims=True)
            ez = np.exp(zz)
            u = ez / ez.sum(axis=-1, keepdims=True)
        else:
            u = np.maximum(z, 0.0)
    return np.ascontiguousarray(u, dtype=np.float32)


# revision 8
# speedup vs baseline: 1.7158x; 1.3789x over previous
"""MemNN (end-to-end memory network) Trainium2 kernel.

The heavy FLOPs are six (B*L, V) @ (V, D) embedding matmuls sharing `facts`
as LHS (A_h = facts @ Wa[h], C_h = facts @ Wc[h]) that fuse into one
(3200, 10000) @ (10000, 1536) matmul independent of the hop recurrence.
Sharding: vocab (contraction) split 8 ways; each core computes a partial
product, the host sums the 8 partials and runs the tiny hop recurrence.

Three measured facts about TRN2's PE drive the design:
 - bf16 streams 2 moving rows/cycle (fp32r only 1), so bf16 matmuls cost
   200 cycles per 400-wide instruction;
 - every matmul instruction is preceded by a ~256-cycle weight load, so a
   pure-bf16 schedule is load-bound (measured 360 cyc/instruction);
 - fp8 + MatmulPerfMode.DoubleRow covers 2 contraction tiles per load at
   the same stream rate, halving load traffic per FLOP.

Schedule per 400-col moving chunk: A-half in bf16 (its noise feeds the
softmax-amplified attention logits; fp8 there fails the 2e-2 tolerance),
C-half in fp8 e4m3 DoubleRow (C only enters through the smooth p-weighted
average).  Loads (~19.7k cyc) then hide under streams (22k cyc).

Two more reductions:
 - A_0 is only ever used as match_0 = A_0 . u_0, and u_0 = sum(question)
   @ Wq depends on nothing else -- so the host computes u_0 and
   g = Wa[0] @ u_0^T (0.7% of the FLOPs, by associativity) and the device
   contracts facts against the 64-wide g instead of the 256-wide Wa[0],
   a 4x saving on that half-hop; the host then reads the (b,b) diagonal
   blocks.  This also makes the device question matmul redundant.
 - Partials are written in bf16, keeping total DMA (14.8 MB in + 8.6 MB
   out per core) under the PE stream time.

C-side fp8 details: facts are mean-shifted (f - 0.5, halving quantization
noise; the exact rank-1 correction att += 0.5*colsum(Wc) is a constant
vector add on the host since p sums to 1) and Wc is pre-scaled by 2^11 so
its ~0.02 entries land in e4m3's normal range (max finite 240).

End-to-end relative error ~8e-3 vs the 2e-2 tolerance, numpy-simulated on
the exact inputs; earlier hardware runs tracked the simulation within 2e-5.
"""

import os

os.environ.setdefault("MYCRO_LOCAL_CACHE", "1")

import ml_dtypes
import numpy as np

import concourse.bass as bass
import concourse.mybir as mybir
import concourse.tile as tile
from concourse.bass_utils import run_bass_kernel_spmd

HOPS, B, L, V, D = 3, 64, 50, 10000, 256
NCORES = 8
BL = B * L                # 3200 moving rows
NA = 2 * D                # 512 bf16 A cols: [Wa1|Wa2]
NC_ = HOPS * D            # 768 fp8 C cols: [Wc0|Wc1|Wc2]
VSH = V // NCORES         # 1250 vocab rows per core
KT = 10                   # contraction tiles of 128 per core
VPAD = KT * 128           # 1280 (zero-padded)
MCH = 400                 # moving-col chunk
WSC = 2048.0              # 2^11 Wc pre-scale for fp8
F32 = mybir.dt.float32
BF16 = mybir.dt.bfloat16
FP8 = mybir.dt.float8e4
NP_BF16 = ml_dtypes.bfloat16
NP_FP8 = ml_dtypes.float8_e4m3
DR = mybir.MatmulPerfMode.DoubleRow

_nc_cache = None
_last_result = None       # BassKernelResults of the most recent run (for profiling)


def _legalize_sync(nc):
    """Split multi-wait sync_info into standalone single-wait EventSemaphores.

    The walrus build in this environment enforces the raw-bass contract of at
    most ONE SyncWait per instruction ("Too many sync wait commands" in
    setupSyncWait otherwise), while Tile attaches every needed wait to the
    consuming instruction.  Hoisting all-but-one wait onto preceding
    InstEventSemaphore instructions on the same engine queue is semantically
    identical: engine queues are in-order, so a preceding wait blocks the
    queue exactly like an attached wait.  Updates are left untouched (they
    fire at completion and cannot be hoisted).
    """
    for func in nc.m.functions:
        for block in func.blocks:
            insts = list(block.instructions)
            out = []
            n = 0
            for inst in insts:
                si = inst.sync_info
                if si is not None and len(si.on_wait) > 1:
                    waits = list(si.on_wait)
                    for w in waits[:-1]:
                        ev = mybir.InstEventSemaphore(
                            name=f"{inst.name}-hoistw{n}", ins=[], outs=[]
                        )
                        n += 1
                        ev.engine = inst.engine
                        ev.sync_info = mybir.SyncInfo(on_wait=[w], on_update=[])
                        nc.register_instruction(ev)
                        out.append(ev)
                    inst.sync_info = mybir.SyncInfo(
                        on_wait=[waits[-1]], on_update=list(si.on_update)
                    )
                out.append(inst)
            if len(out) != len(insts):
                block.instructions = out
    return nc


_WIDTHS = [MCH] * (BL // MCH)
_STARTS = [sum(_WIDTHS[:i]) for i in range(len(_WIDTHS))]
assert sum(_WIDTHS) == BL


def _build(reps=1):
    """Build the SPMD device program.

    reps>1 repeats the main loop body (same data, same output addresses) --
    used only by the benchmark harness to measure device time differentially
    (per-call dispatch noise over the axon tunnel is ~ms, device time is
    ~100 us, so wall-clocking one launch cannot resolve it).
    """
    nc = bass.Bass(trn_type="TRN2")
    facts_b = nc.dram_tensor("facts_b", [VPAD, BL], BF16, kind="ExternalInput")
    facts_8 = nc.dram_tensor("facts_8", [VPAD, BL], FP8, kind="ExternalInput")
    wa_b = nc.dram_tensor("wa_b", [VPAD, NA], BF16, kind="ExternalInput")
    g_b = nc.dram_tensor("g_b", [VPAD, B], BF16, kind="ExternalInput")
    wc_8 = nc.dram_tensor("wc_8", [VPAD, NC_], FP8, kind="ExternalInput")
    pac_b = nc.dram_tensor("pac_b", [NA + NC_, BL], BF16, kind="ExternalOutput")
    pm0 = nc.dram_tensor("pm0", [B, BL], BF16, kind="ExternalOutput")

    fbr = facts_b.rearrange("(k p) n -> p k n", p=128)
    f8r = facts_8.rearrange("(k p) n -> p k n", p=128)
    war = wa_b.rearrange("(k p) n -> p k n", p=128)
    gr = g_b.rearrange("(k p) n -> p k n", p=128)
    wcr = wc_8.rearrange("(k p) n -> p k n", p=128)
    wmax = max(_WIDTHS)
    NNA = NA // 128           # 4 bf16 A n-tiles
    NNC = NC_ // 128          # 6 fp8 C n-tiles

    with (
        tile.TileContext(nc) as tc,
        tc.tile_pool(name="wpool", bufs=1) as wpool,
        tc.tile_pool(name="xbpool", bufs=3) as xbpool,
        tc.tile_pool(name="x8pool", bufs=3) as x8pool,
        tc.tile_pool(name="opool", bufs=6) as opool,
        tc.tile_pool(name="pspool", bufs=7, space="PSUM") as pspool,
    ):
        # Prologue DMA order: the first matmul group's deps (g + first bf16
        # facts chunk), then the remaining weights, then the fp8 chunk.
        gt = wpool.tile([128, KT, B], BF16)
        nc.sync.dma_start(gt[:], gr)
        xbs = {}
        x8s = {}
        xbs[0] = xbpool.tile(
            [128, KT, _WIDTHS[0]], BF16, tag="xb", name="xb",
            padded_shape=[128, KT, wmax],
        )
        nc.sync.dma_start(xbs[0][:], fbr[:, :, 0 : _WIDTHS[0]])
        wat = wpool.tile([128, KT, NA], BF16)
        for off in range(0, NA, 256):
            nc.sync.dma_start(wat[:, :, off : off + 256], war[:, :, off : off + 256])
        wct = wpool.tile([128, KT, NC_], FP8)
        for off in range(0, NC_, 384):
            nc.sync.dma_start(wct[:, :, off : off + 384], wcr[:, :, off : off + 384])
        x8s[0] = x8pool.tile(
            [128, KT, _WIDTHS[0]], FP8, tag="x8", name="x8",
            padded_shape=[128, KT, wmax],
        )
        nc.sync.dma_start(x8s[0][:], f8r[:, :, 0 : _WIDTHS[0]])

        def get_xt(mi, xs, pool, dt, rr, tg):
            if mi not in xs:
                xs[mi] = pool.tile(
                    [128, KT, _WIDTHS[mi]], dt, tag=tg, name=tg,
                    padded_shape=[128, KT, wmax],
                )
                nc.sync.dma_start(
                    xs[mi][:], rr[:, :, _STARTS[mi] : _STARTS[mi] + _WIDTHS[mi]]
                )
            return xs[mi]

        def drain(ps, dst, row0, nrows, mi):
            ot = opool.tile(
                [nrows, _WIDTHS[mi]], BF16, tag="ot", name="ot",
                padded_shape=[128, wmax],
            )
            nc.vector.tensor_copy(ot[:], ps[:])
            nc.sync.dma_start(
                dst[row0 : row0 + nrows, _STARTS[mi] : _STARTS[mi] + _WIDTHS[mi]],
                ot[:],
            )

        # Main fused matmul per chunk: the 64-wide match0 group, 4 bf16 A
        # n-tiles, then 6 fp8 DoubleRow C n-tiles.
        for _ in range(reps):
            for mi in range(len(_WIDTHS)):
                xb = get_xt(mi, xbs, xbpool, BF16, fbr, "xb")
                x8 = get_xt(mi, x8s, x8pool, FP8, f8r, "x8")
                psm = pspool.tile(
                    [B, _WIDTHS[mi]], F32, tag="ps", name="ps",
                    padded_shape=[128, wmax],
                )
                for k in range(KT):
                    nc.tensor.matmul(
                        psm[:], gt[:, k, :], xb[:, k, :],
                        start=(k == 0), stop=(k == KT - 1),
                    )
                drain(psm, pm0, 0, B, mi)
                for n in range(NNA):
                    ps = pspool.tile(
                        [128, _WIDTHS[mi]], F32, tag="ps", name="ps",
                        padded_shape=[128, wmax],
                    )
                    for k in range(KT):
                        nc.tensor.matmul(
                            ps[:],
                            wat[:, k, n * 128 : (n + 1) * 128],
                            xb[:, k, :],
                            start=(k == 0),
                            stop=(k == KT - 1),
                        )
                    drain(ps, pac_b, n * 128, 128, mi)
                for n in range(NNC):
                    ps = pspool.tile(
                        [128, _WIDTHS[mi]], F32, tag="ps", name="ps",
                        padded_shape=[128, wmax],
                    )
                    for t in range(KT // 2):
                        nc.tensor.matmul(
                            ps[:],
                            wct[:, 2 * t : 2 * t + 2, n * 128 : (n + 1) * 128],
                            x8[:, 2 * t : 2 * t + 2, :],
                            start=(t == 0),
                            stop=(t == KT // 2 - 1),
                            perf_mode=DR,
                        )
                    drain(ps, pac_b, NA + n * 128, 128, mi)
            xbs.clear()
            x8s.clear()
    return _legalize_sync(nc)


def _shard_inputs(facts, question, Wq, Wa, Wc):
    fx = np.ascontiguousarray(facts, dtype=np.float32).reshape(BL, V)
    fxb = fx.astype(NP_BF16)
    fx8 = (fx - np.float32(0.5)).astype(NP_FP8)
    qx = np.asarray(question, dtype=np.float32).sum(axis=1)  # (B, V) bag-of-words
    Wq = np.asarray(Wq, dtype=np.float32)
    Wa = np.asarray(Wa, dtype=np.float32)
    Wc = np.asarray(Wc, dtype=np.float32)
    u0 = qx @ Wq                                  # (B, D) exact, on host
    g = (Wa[0] @ u0.T).astype(NP_BF16)            # (V, B): match0 projection
    wa12 = np.concatenate([Wa[1], Wa[2]], axis=1).astype(NP_BF16)
    wc8 = (
        np.concatenate([Wc[0], Wc[1], Wc[2]], axis=1) * np.float32(WSC)
    ).astype(NP_FP8)

    in_maps = []
    for c in range(NCORES):
        sl = slice(c * VSH, (c + 1) * VSH)
        fb = np.zeros((VPAD, BL), NP_BF16)
        fb[:VSH] = fxb[:, sl].T
        f8 = np.zeros((VPAD, BL), NP_FP8)
        f8[:VSH] = fx8[:, sl].T
        wab = np.zeros((VPAD, NA), NP_BF16)
        wab[:VSH] = wa12[sl]
        gb = np.zeros((VPAD, B), NP_BF16)
        gb[:VSH] = g[sl]
        wcb = np.zeros((VPAD, NC_), NP_FP8)
        wcb[:VSH] = wc8[sl]
        in_maps.append(
            {"facts_b": fb, "facts_8": f8, "wa_b": wab, "g_b": gb, "wc_8": wcb}
        )
    return in_maps, u0


def _wait_for_devices(min_wait_attempts=10):
    """The axon terminal occasionally reports a transient bad topology
    ("terminal has 1 core"); poll until all 8 NeuronCores are visible."""
    import time as _time

    import jax

    for attempt in range(min_wait_attempts):
        try:
            if len(jax.devices()) >= NCORES:
                return
        except Exception:  # noqa: BLE001 - backend init failure is retryable
            try:
                jax.clear_backends()
            except Exception:  # noqa: BLE001
                pass
        _time.sleep(15.0)
    # fall through: let the run itself raise a descriptive error


def _run_with_retries(nc, in_maps, attempts=4):
    """run_bass_kernel_spmd with retries: the axon terminal occasionally
    reports transient failures (device wedged / NRT_EXEC_UNIT_UNRECOVERABLE /
    temporary topology glitches) that succeed on re-dispatch."""
    import time as _time

    last_exc = None
    for attempt in range(attempts):
        try:
            return run_bass_kernel_spmd(nc, in_maps, list(range(NCORES)))
        except Exception as e:  # noqa: BLE001 - retry any runtime failure
            last_exc = e
            if attempt < attempts - 1:
                _time.sleep(10.0 * (attempt + 1))
                _wait_for_devices(min_wait_attempts=4)
    raise last_exc


def kernel(facts, question, Wq, Wa, Wc, Ww, bw):
    global _nc_cache, _last_result
    _wait_for_devices(min_wait_attempts=8)
    in_maps, u0 = _shard_inputs(facts, question, Wq, Wa, Wc)
    if _nc_cache is None:
        _nc_cache = _build()
    _last_result = _run_with_retries(_nc_cache, in_maps)
    res = _last_result.results

    # Unshard: sum the 8 bf16 partial products of the vocab-sharded matmul.
    ac = res[0]["pac_b"].astype(np.float32)
    m0 = res[0]["pm0"].astype(np.float32)
    for r in res[1:]:
        ac += r["pac_b"].astype(np.float32)
        m0 += r["pm0"].astype(np.float32)

    Wc = np.asarray(Wc, dtype=np.float32)
    colsum_wc = Wc.sum(axis=1)  # (HOPS, D): exact rank-1 shift correction

    # match0 = facts @ (Wa0 @ u0^T), diagonal (b, b) blocks of pm0.
    match0 = m0.reshape(B, B, L)[np.arange(B), np.arange(B)]

    # Sequential hop recurrence (tiny: ~30 MFLOP vs 98.3 GFLOP on device).
    Ww = np.asarray(Ww, dtype=np.float32)
    bw = np.asarray(bw, dtype=np.float32)
    u = u0
    for h in range(HOPS):
        C = ac[NA + h * D : NA + (h + 1) * D].reshape(D, B, L)
        if h == 0:
            match = match0
        else:
            A = ac[(h - 1) * D : h * D].reshape(D, B, L)
            match = np.einsum("dbl,bd->bl", A, u)
        mm = match - match.max(axis=-1, keepdims=True)
        e = np.exp(mm)
        p = e / e.sum(axis=-1, keepdims=True)
        # C partials carry the 2^11 fp8 pre-scale; p sums to 1, so the
        # mean-shift correction is a constant vector add.
        att = np.einsum("bl,dbl->bd", p, C) * np.float32(1.0 / WSC)
        att += np.float32(0.5) * colsum_wc[h]
        z = (u + att) @ Ww[h] + bw[h]
        if h == HOPS - 1:
            zz = z - z.max(axis=-1, keepdims=True)
            ez = np.exp(zz)
            u = ez / ez.sum(axis=-1, keepdims=True)
        else:
            u = np.maximum(z, 0.0)
    return np.ascontiguousarray(u, dtype=np.float32)
